# revision 19
# baseline (speedup 1.0000x reference)
"""Trainium2 Bass kernel for the CNN-MAD per-class DTW transport cost.

Math (reference):
  mat_cost[n, j] = C1[n] + C2[c_n, j] - 2*C3[n, j],  c_n = classes[n]
    C1[n]    = sum_t rowsum[c_n, t] * ||X[n,t,:]||^2
    C2[c, j] = sum_p colsum[c, p] * ||Y[j,p,:]||^2
    C3[n, j] = sum_{p,d} (sum_t pi[c_n,t,p] X[n,t,d]) * Y[j,p,d]

Sharding 4x2: core k = (g, h) with g = k>>1 (class group: classes 2g, 2g+1,
each padded to 144 sample slots) and h = k&1 (Y half, 512 rows). The host
only regroups / transposes / dtype-casts; all arithmetic is on device.

Precision: inputs cast to fp8 e4m3 on host (X, Y absmax ~5.4; pi is 0/1 so
exact). Heavy matmuls run fp8 DoubleRow (K=256/instr) into f32 PSUM. The
large C1/C2 terms ride fp16 paths (C1 via a K=1 ones matmul into PSUM, C2
transposed [j, c] added as a per-partition bias during the fp16 output
evacuation). End-to-end rel err ~1e-3 vs the 2e-2 gate.

Device layout per core (C3 contraction k=(pt,d,pp), t=(tt,tp)):
  pis [tp 128, (c 2, tt 2, p 256) | (c 2, pt 2, t 256)]  fp8 (pi and pi^T)
  xk  [tp 128, d 8, tt 2, n 288]  fp8   xk[tp,d,tt,n] = X[n, tt*128+tp, d]
  yt  [pp 128, kc 16, j 512]      fp8   kc=(pt,d): yt = Y[j, pt*128+pp, d]
  crps: colsum^T / rowsum^T via 8 DoubleRow ones-matmuls (one PSUM bank)
  XW:  per (pt,d) granule, 2 class DoubleRows; evac * -2 -> xwt fp8
  xsq/ysq: elementwise fp8 squares split across ACT/DVE/Pool
  C1:  DoubleRow rw8.T @ xsq -> psum [2, 288] -> fp16 + per-class select
       -> c1row [1, 288]; added into each C3 psum by a K=1 ones matmul
  C2t: DoubleRow ysq.T @ cs2 -> psum [j 128, c 2] per jt (transposed C2)
  C3:  kc-pair DoubleRow yt.T @ xwt into 4 psum banks [j 128, n 288]
  out: fp16 evac with per-partition bias C2t[j, c], two DMAs out
"""

import sys

sys.path.insert(0, "/opt/trn_rl_repo")

import numpy as np
import ml_dtypes

N, NY, T, TP, D, C = 1024, 1024, 256, 256, 8, 8
NCORES = 8
G, H = 4, 2          # class groups x Y halves
CPC = 144            # per-class sample capacity (max count is 144)
CAP = 2 * CPC        # 288 sample columns per core
NYH = NY // H        # 512
KC = 16              # 128-row contraction chunks of C3, kc = (pt, d)
JT = NYH // 128      # 4 output row tiles

FP8 = ml_dtypes.float8_e4m3

_cache = {}


def _build():
    import concourse.bacc as bacc
    import concourse.mybir as mybir
    import concourse.tile as tile

    f8 = mybir.dt.float8e4
    f16 = mybir.dt.float16
    f32 = mybir.dt.float32
    DR = mybir.MatmulPerfMode.DoubleRow
    Ident = mybir.ActivationFunctionType.Identity
    nc = bacc.Bacc("TRN2", target_bir_lowering=False, debug=False, num_devices=NCORES)

    pis_d = nc.dram_tensor("pis", [128, 2 * 2 * 2 * TP], f8, kind="ExternalInput")
    xk_d = nc.dram_tensor("xk", [128, KC * CAP], f8, kind="ExternalInput")
    yt_d = nc.dram_tensor("yt", [128, KC * NYH], f8, kind="ExternalInput")
    out_d = nc.dram_tensor("outp", [NYH, CAP], f16, kind="ExternalOutput")

    with tile.TileContext(nc) as tc:
        with (
            tc.tile_pool(name="const", bufs=1) as pc,
            tc.tile_pool(name="xin", bufs=1) as px,
            tc.tile_pool(name="yin", bufs=1) as py,
            tc.tile_pool(name="psA", bufs=6, space="PSUM") as psA,
            tc.tile_pool(name="psB", bufs=2, space="PSUM") as psB,
        ):
            # ---- input DMAs on the SP HWDGE queue ----
            pis = pc.tile([128, 2, 2, 2, TP], f8, tag="pis")
            pisv = pis_d.rearrange("l (w c u p) -> l w c u p", w=2, c=2, u=2)
            pi = pis[:, 0, :, :, :]    # [tp, c, tt, p]
            piT = pis[:, 1, :, :, :]   # [pp, c, pt, t]

            xk = px.tile([128, D, 2, CAP], f8, tag="xk")
            xkv = xk_d.rearrange("l (d u n) -> l d u n", d=D, u=2)
            yt = py.tile([128, KC, NYH], f8, tag="yt")
            ytv = yt_d.rearrange("l (k j) -> l k j", k=KC)
            nc.sync.dma_start(pis[:], pisv)
            nc.sync.dma_start(xk[:, 0:4, :, :], xkv[:, 0:4, :, :])
            nc.sync.dma_start(xk[:, 4:8, :, :], xkv[:, 4:8, :, :])
            for q in range(4):
                nc.sync.dma_start(yt[:, 4 * q : 4 * q + 4, :], ytv[:, 4 * q : 4 * q + 4, :])

            # ---- small constants (Pool; everything tiny and early) ----
            ones8 = pc.tile([128, 2, 1], f8, tag="ones8")
            nc.gpsimd.memset(ones8[:], 1.0)
            ones16 = pc.tile([1, 128], f16, tag="ones16")
            nc.gpsimd.memset(ones16[:], 1.0)

            # ---- colsum^T (c,pt) and rowsum^T (c,tt) via ones DoubleRows ----
            crps = psB.tile([128, 8], f32, tag="psB", name="crps")
            for c in range(2):
                for pt in range(2):
                    nc.tensor.matmul(
                        crps[:, 2 * c + pt : 2 * c + pt + 1],
                        pi[:, c, :, pt * 128 : (pt + 1) * 128],
                        ones8[:],
                        start=True, stop=True, perf_mode=DR,
                        skip_group_check=True,
                    )
            for c in range(2):
                for tt in range(2):
                    nc.tensor.matmul(
                        crps[:, 4 + 2 * c + tt : 5 + 2 * c + tt],
                        piT[:, c, :, tt * 128 : (tt + 1) * 128],
                        ones8[:],
                        start=True, stop=True, perf_mode=DR,
                        skip_group_check=True,
                    )
            cs2 = pc.tile([128, 2, 2, 2], f8, tag="cs2")   # [pp, dup, pt, c]
            csv = crps[:, 0:4].rearrange("l (c pt) -> l pt c", c=2)
            nc.vector.tensor_copy(cs2[:, 0, :, :], csv)
            nc.vector.tensor_copy(cs2[:, 1, :, :], csv)
            rw8 = pc.tile([128, 2, 2], f8, tag="rw8")      # [tp, tt, c]
            nc.gpsimd.tensor_copy(rw8[:], crps[:, 4:8].rearrange("l (c u) -> l u c", c=2))

            xwt = px.tile([128, KC, CAP], f8, tag="xwt")
            xsq = px.tile([128, D, 2, CAP], f8, tag="xsq")
            ysq = py.tile([128, KC, NYH], f8, tag="ysq")

            def xw_granule(pt, d, evac):
                g = psA.tile([128, CAP], f32, tag="psA", name=f"xw{pt}_{d}")
                for cl in range(2):
                    nc.tensor.matmul(
                        g[:, cl * CPC : (cl + 1) * CPC],
                        pi[:, cl, :, pt * 128 : (pt + 1) * 128],
                        xk[:, d, :, cl * CPC : (cl + 1) * CPC],
                        start=True, stop=True, perf_mode=DR,
                        skip_group_check=True,
                    )
                kc = pt * D + d
                evac(xwt[:, kc, :], g[:], -2.0)

            dve_m, act_m, pool_m = (nc.vector.tensor_scalar_mul, nc.scalar.mul,
                                    nc.gpsimd.tensor_scalar_mul)

            # ---- xk half 1 (d 0-3): XW kc 0-3, 8-11 (Pool-heavy evacs),
            # xsq half as one DVE quad ----
            for d in range(4):
                xw_granule(0, d, (dve_m, pool_m, pool_m, pool_m)[d])
            for d in range(4):
                xw_granule(1, d, (pool_m, act_m, pool_m, pool_m)[d])
            nc.vector.tensor_mul(xsq[:, 0:4, :, :], xk[:, 0:4, :, :], xk[:, 0:4, :, :])

            # ---- xk half 2 (d 4-7): xsq quad on ACT first (C1 chain) ----
            nc.scalar.square(xsq[:, 4:8, :, :], xk[:, 4:8, :, :])
            for d in range(4, 8):
                xw_granule(0, d, (pool_m, pool_m, act_m, act_m)[d - 4])
            for d in range(4, 8):
                xw_granule(1, d, (pool_m, act_m, pool_m, pool_m)[d - 4])

            # ---- C1 per class: DoubleRows -> two [1, CAP] psum rows ----
            c1ps = [
                psA.tile([1, CAP], f32, tag="psA", name=f"c1ps{cl}")
                for cl in range(2)
            ]
            for d in range(D):
                for cl in range(2):
                    nc.tensor.matmul(
                        c1ps[cl][:], rw8[:, :, cl : cl + 1], xsq[:, d, :, :],
                        start=(d == 0), stop=(d == D - 1), perf_mode=DR,
                        skip_group_check=True,
                    )
            c1a = pc.tile([1, CAP], f16, tag="c1a")
            c1b = pc.tile([1, CAP], f16, tag="c1b")
            nc.scalar.copy(c1a[:], c1ps[0][:])
            nc.gpsimd.tensor_copy(c1b[:], c1ps[1][:])

            # ---- streaming: remaining ysq + C3/C2t per kc pair ----
            c2ps = psB.tile([128, JT, 2], f32, tag="psB", name="c2ps")
            c3ps = [
                psA.tile([128, CAP], f32, tag="psA", name=f"c3_{jt}")
                for jt in range(JT)
            ]

            def c3_r(r, start):
                for jt in range(JT):
                    nc.tensor.matmul(
                        c3ps[jt][:],
                        yt[:, 2 * r : 2 * r + 2, jt * 128 : (jt + 1) * 128],
                        xwt[:, 2 * r : 2 * r + 2, :],
                        start=start, stop=False, perf_mode=DR,
                        skip_group_check=True,
                    )

            def c2_r(r, start, stop):
                pt = r // 4
                for jt in range(JT):
                    nc.tensor.matmul(
                        c2ps[:, jt, :],
                        ysq[:, 2 * r : 2 * r + 2, jt * 128 : (jt + 1) * 128],
                        cs2[:, :, pt, :],
                        start=start, stop=stop, perf_mode=DR,
                        skip_group_check=True,
                    )

            # ysq as chunk-aligned quad ops, alternating DVE/ACT
            nc.vector.tensor_mul(ysq[:, 0:4, :], yt[:, 0:4, :], yt[:, 0:4, :])
            c3_r(0, True); c2_r(0, True, False)
            c3_r(1, False); c2_r(1, False, False)

            nc.scalar.square(ysq[:, 4:8, :], yt[:, 4:8, :])
            c3_r(2, False); c2_r(2, False, False)
            c3_r(3, False); c2_r(3, False, False)

            nc.vector.tensor_mul(ysq[:, 8:12, :], yt[:, 8:12, :], yt[:, 8:12, :])
            c3_r(4, False); c2_r(4, False, False)
            c3_r(5, False); c2_r(5, False, False)

            nc.scalar.square(ysq[:, 12:16, :], yt[:, 12:16, :])
            c3_r(6, False); c2_r(6, False, False)
            c3_r(7, False)
            # close the C3 groups with two K=1 fp16 C1 augmentations each
            # (these depend only on xk, so the psums close well before C2t)
            for jt in range(JT):
                nc.tensor.matmul(
                    c3ps[jt][:, 0:CPC], ones16[:], c1a[0:1, 0:CPC],
                    start=False, stop=False, skip_group_check=True,
                )
                nc.tensor.matmul(
                    c3ps[jt][:, CPC:CAP], ones16[:], c1b[0:1, CPC:CAP],
                    start=False, stop=True, skip_group_check=True,
                )
            c2_r(7, False, True)

            # C2t psum -> SBUF f32 (one small op)
            c2sb = pc.tile([128, JT, 2], f32, tag="c2sb")
            nc.gpsimd.tensor_copy(c2sb[:], c2ps[:])

            # ---- out: fp16 evac with per-partition C2 bias, 2 DMAs ----
            osb = py.tile([128, JT, CAP], f16, tag="osb")
            odv = out_d.rearrange("(jt l) n -> l jt n", l=128)

            def bias_evac(eng, jt, cl):
                dst = osb[:, jt, cl * CPC : (cl + 1) * CPC]
                src = c3ps[jt][:, cl * CPC : (cl + 1) * CPC]
                b = c2sb[:, jt, cl : cl + 1]
                if eng is nc.scalar:
                    eng.activation(dst, src, Ident, bias=b)
                else:
                    eng.tensor_scalar_add(dst, src, b)

            bias_evac(nc.vector, 0, 0); bias_evac(nc.scalar, 0, 1)
            bias_evac(nc.gpsimd, 1, 0); bias_evac(nc.vector, 1, 1)
            nc.sync.dma_start(odv[:, 0:2, :], osb[:, 0:2, :])
            bias_evac(nc.scalar, 2, 0); bias_evac(nc.gpsimd, 2, 1)
            bias_evac(nc.vector, 3, 0); bias_evac(nc.scalar, 3, 1)
            nc.scalar.dma_start(odv[:, 2:4, :], osb[:, 2:4, :])

    nc.compile()
    return nc


def kernel(X, Y, pi_dtw, classes):
    from concourse.bass_utils import run_bass_kernel_spmd

    X = np.asarray(X, dtype=np.float32)
    Y = np.asarray(Y, dtype=np.float32)
    pi_dtw = np.asarray(pi_dtw, dtype=np.float32)
    classes = np.asarray(classes).astype(np.int64)

    if "nc" not in _cache:
        _cache["nc"] = _build()
    nc = _cache["nc"]

    X8 = X.astype(FP8)
    Y8 = Y.astype(FP8)
    pi8 = pi_dtw.astype(FP8)
    idx = [np.nonzero(classes == c)[0] for c in range(C)]
    assert max(len(i) for i in idx) <= CPC, "class count exceeds capacity"

    # yt per Y half: [pp, (pt, d), j]
    yts = []
    for h in range(H):
        yh = Y8[h * NYH : (h + 1) * NYH]          # [j, p, d]
        a = yh.reshape(NYH, 2, 128, D).transpose(2, 1, 3, 0)  # [pp, pt, d, j]
        yts.append(np.ascontiguousarray(a.reshape(128, KC * NYH)))

    in_maps = []
    for k in range(NCORES):
        g, h = k >> 1, k & 1
        c0, c1 = 2 * g, 2 * g + 1
        xg = np.zeros((CAP, T, D), dtype=FP8)
        xg[0 : len(idx[c0])] = X8[idx[c0]]
        xg[CPC : CPC + len(idx[c1])] = X8[idx[c1]]
        # xk: [tp, d, tt, n]
        a = xg.reshape(CAP, 2, 128, D).transpose(2, 3, 1, 0)
        xk = np.ascontiguousarray(a.reshape(128, KC * CAP))
        # pis: pi [tp, c, tt, p] ++ piT [pp, c, pt, t]
        pg = pi8[[c0, c1]]                         # [c, t, p]
        b = pg.reshape(2, 2, 128, TP).transpose(2, 0, 1, 3)          # [tp,c,tt,p]
        bt = pg.reshape(2, TP, 2, 128).transpose(3, 0, 2, 1)         # [pp,c,pt,t]
        pik = np.concatenate(
            [b.reshape(128, -1), bt.reshape(128, -1)], axis=1
        )
        in_maps.append({"pis": np.ascontiguousarray(pik), "xk": xk, "yt": yts[h]})

    res = run_bass_kernel_spmd(nc, in_maps, core_ids=list(range(NCORES)))

    out = np.empty((N, NY), dtype=np.float32)
    for k in range(NCORES):
        g, h = k >> 1, k & 1
        blk = np.asarray(res.results[k]["outp"]).astype(np.float32)  # [j, n]
        jsel = slice(h * NYH, (h + 1) * NYH)
        c0, c1 = 2 * g, 2 * g + 1
        out[idx[c0], jsel] = blk[:, 0 : len(idx[c0])].T
        out[idx[c1], jsel] = blk[:, CPC : CPC + len(idx[c1])].T
    return out


# revision 22
# speedup vs baseline: 1.1031x; 1.1031x over previous
"""Trainium2 Bass kernel for the CNN-MAD per-class DTW transport cost.

Math (reference):
  mat_cost[n, j] = C1[n] + C2[c_n, j] - 2*C3[n, j],  c_n = classes[n]
    C1[n]    = sum_t rowsum[c_n, t] * ||X[n,t,:]||^2
    C2[c, j] = sum_p colsum[c, p] * ||Y[j,p,:]||^2
    C3[n, j] = sum_{p,d} (sum_t pi[c_n,t,p] X[n,t,d]) * Y[j,p,d]

Sharding 4x2: core k = (g, h) with g = k>>1 (class group: classes 2g, 2g+1,
each padded to 144 sample slots) and h = k&1 (Y half, 512 rows). The host
only regroups / transposes / dtype-casts; all arithmetic is on device.

Precision: inputs cast to fp8 e4m3 on host (X, Y absmax ~5.4; pi is 0/1 so
exact). Heavy matmuls run fp8 DoubleRow (K=256/instr) into f32 PSUM. The
large C1/C2 terms ride fp16 paths (C1 via a K=1 ones matmul into PSUM, C2
transposed [j, c] added as a per-partition bias during the fp16 output
evacuation). End-to-end rel err ~1e-3 vs the 2e-2 gate.

Device layout per core (C3 contraction k=(pt,d,pp), t=(tt,tp)):
  pis [tp 128, (c 2, tt 2, p 256) | (c 2, pt 2, t 256)]  fp8 (pi and pi^T)
  xk  [tp 128, d 8, tt 2, n 288]  fp8   xk[tp,d,tt,n] = X[n, tt*128+tp, d]
  yt  [pp 128, kc 16, j 512]      fp8   kc=(pt,d): yt = Y[j, pt*128+pp, d]
  crps: colsum^T / rowsum^T via 8 DoubleRow ones-matmuls (one PSUM bank)
  XW:  per (pt,d) granule, 2 class DoubleRows; evac * -2 -> xwt fp8
  xsq/ysq: elementwise fp8 squares split across ACT/DVE/Pool
  C1:  DoubleRow rw8.T @ xsq -> psum [2, 288] -> fp16 + per-class select
       -> c1row [1, 288]; added into each C3 psum by a K=1 ones matmul
  C2t: DoubleRow ysq.T @ cs2 -> psum [j 128, c 2] per jt (transposed C2)
  C3:  kc-pair DoubleRow yt.T @ xwt into 4 psum banks [j 128, n 288]
  out: fp16 evac with per-partition bias C2t[j, c], two DMAs out
"""

import sys

sys.path.insert(0, "/opt/trn_rl_repo")

import numpy as np
import ml_dtypes

N, NY, T, TP, D, C = 1024, 1024, 256, 256, 8, 8
NCORES = 8
G, H = 4, 2          # class groups x Y halves
CPC = 144            # per-class sample capacity (max count is 144)
CAP = 2 * CPC        # 288 sample columns per core
NYH = NY // H        # 512
KC = 16              # 128-row contraction chunks of C3, kc = (pt, d)
JT = NYH // 128      # 4 output row tiles

FP8 = ml_dtypes.float8_e4m3

_cache = {}


def _build():
    import concourse.bacc as bacc
    import concourse.mybir as mybir
    import concourse.tile as tile

    f8 = mybir.dt.float8e4
    f16 = mybir.dt.float16
    f32 = mybir.dt.float32
    DR = mybir.MatmulPerfMode.DoubleRow
    Ident = mybir.ActivationFunctionType.Identity
    nc = bacc.Bacc("TRN2", target_bir_lowering=False, debug=False, num_devices=NCORES)

    pis_d = nc.dram_tensor("pis", [128, 2 * 2 * 2 * TP], f8, kind="ExternalInput")
    xk_d = nc.dram_tensor("xk", [128, KC * CAP], f8, kind="ExternalInput")
    yt_d = nc.dram_tensor("yt", [128, KC * NYH], f8, kind="ExternalInput")
    out_d = nc.dram_tensor("outp", [NYH, CAP], f16, kind="ExternalOutput")

    with tile.TileContext(nc) as tc:
        with (
            tc.tile_pool(name="const", bufs=1) as pc,
            tc.tile_pool(name="xin", bufs=1) as px,
            tc.tile_pool(name="yin", bufs=1) as py,
            tc.tile_pool(name="psA", bufs=6, space="PSUM") as psA,
            tc.tile_pool(name="psB", bufs=2, space="PSUM") as psB,
        ):
            # ---- input DMAs on the SP HWDGE queue ----
            pis = pc.tile([128, 2, 2, 2, TP], f8, tag="pis")
            pisv = pis_d.rearrange("l (w c u p) -> l w c u p", w=2, c=2, u=2)
            pi = pis[:, 0, :, :, :]    # [tp, c, tt, p]
            piT = pis[:, 1, :, :, :]   # [pp, c, pt, t]

            xk = px.tile([128, D, 2, CAP], f8, tag="xk")
            xkv = xk_d.rearrange("l (d u n) -> l d u n", d=D, u=2)
            yt = py.tile([128, KC, NYH], f8, tag="yt")
            ytv = yt_d.rearrange("l (k j) -> l k j", k=KC)
            nc.sync.dma_start(pis[:], pisv)
            nc.sync.dma_start(xk[:, 0:4, :, :], xkv[:, 0:4, :, :])
            nc.sync.dma_start(xk[:, 4:8, :, :], xkv[:, 4:8, :, :])
            for q in range(4):
                nc.sync.dma_start(yt[:, 4 * q : 4 * q + 4, :], ytv[:, 4 * q : 4 * q + 4, :])

            # ---- small constants (Pool; everything tiny and early) ----
            ones8 = pc.tile([128, 2, 1], f8, tag="ones8")
            nc.gpsimd.memset(ones8[:], 1.0)
            ones16 = pc.tile([1, 128], f16, tag="ones16")
            nc.gpsimd.memset(ones16[:], 1.0)

            # ---- colsum^T (c,pt) and rowsum^T (c,tt) via ones DoubleRows ----
            crps = psB.tile([128, 8], f32, tag="psB", name="crps")
            for c in range(2):
                for pt in range(2):
                    nc.tensor.matmul(
                        crps[:, 2 * c + pt : 2 * c + pt + 1],
                        pi[:, c, :, pt * 128 : (pt + 1) * 128],
                        ones8[:],
                        start=True, stop=True, perf_mode=DR,
                        skip_group_check=True,
                    )
            for c in range(2):
                for tt in range(2):
                    nc.tensor.matmul(
                        crps[:, 4 + 2 * c + tt : 5 + 2 * c + tt],
                        piT[:, c, :, tt * 128 : (tt + 1) * 128],
                        ones8[:],
                        start=True, stop=True, perf_mode=DR,
                        skip_group_check=True,
                    )
            cs2 = pc.tile([128, 2, 2, 2], f8, tag="cs2")   # [pp, dup, pt, c]
            csv = crps[:, 0:4].rearrange("l (c pt) -> l pt c", c=2)
            nc.vector.tensor_copy(cs2[:, 0, :, :], csv)
            nc.vector.tensor_copy(cs2[:, 1, :, :], csv)
            rw8 = pc.tile([128, 2, 2], f8, tag="rw8")      # [tp, tt, c]
            nc.vector.tensor_copy(rw8[:], crps[:, 4:8].rearrange("l (c u) -> l u c", c=2))

            xwt = px.tile([128, KC, CAP], f8, tag="xwt")
            xsq = px.tile([128, D, 2, CAP], f8, tag="xsq")
            ysq = py.tile([128, KC, NYH], f8, tag="ysq")

            def xw_granule(pt, d, evac):
                g = psA.tile([128, CAP], f32, tag="psA", name=f"xw{pt}_{d}")
                for cl in range(2):
                    nc.tensor.matmul(
                        g[:, cl * CPC : (cl + 1) * CPC],
                        pi[:, cl, :, pt * 128 : (pt + 1) * 128],
                        xk[:, d, :, cl * CPC : (cl + 1) * CPC],
                        start=True, stop=True, perf_mode=DR,
                        skip_group_check=True,
                    )
                kc = pt * D + d
                evac(xwt[:, kc, :], g[:], -2.0)

            dve_m, act_m, pool_m = (nc.vector.tensor_scalar_mul, nc.scalar.mul,
                                    nc.gpsimd.tensor_scalar_mul)

            # ---- xk half 1 (d 0-3): XW kc 0-3, 8-11 (Pool-heavy evacs),
            # xsq half as one DVE quad ----
            for d in range(4):
                xw_granule(0, d, (dve_m, pool_m, pool_m, pool_m)[d])
            for d in range(4):
                xw_granule(1, d, (pool_m, act_m, pool_m, pool_m)[d])
            nc.vector.tensor_mul(xsq[:, 0:4, :, :], xk[:, 0:4, :, :], xk[:, 0:4, :, :])

            # ---- xk half 2 (d 4-7): xsq quad on ACT first (C1 chain) ----
            nc.scalar.square(xsq[:, 4:8, :, :], xk[:, 4:8, :, :])
            for d in range(4, 8):
                xw_granule(0, d, (pool_m, pool_m, act_m, act_m)[d - 4])
            for d in range(4, 8):
                xw_granule(1, d, (pool_m, act_m, pool_m, pool_m)[d - 4])

            # ---- C1 per class: DoubleRows -> two [1, CAP] psum rows ----
            c1ps = [
                psA.tile([1, CAP], f32, tag="psA", name=f"c1ps{cl}")
                for cl in range(2)
            ]
            for d in range(D):
                for cl in range(2):
                    nc.tensor.matmul(
                        c1ps[cl][:], rw8[:, :, cl : cl + 1], xsq[:, d, :, :],
                        start=(d == 0), stop=(d == D - 1), perf_mode=DR,
                        skip_group_check=True,
                    )
            c1a = pc.tile([1, CAP], f16, tag="c1a")
            c1b = pc.tile([1, CAP], f16, tag="c1b")
            nc.gpsimd.tensor_copy(c1a[:], c1ps[0][:])
            nc.gpsimd.tensor_copy(c1b[:], c1ps[1][:])

            # ---- streaming: remaining ysq + C3/C2t per kc pair ----
            c2ps = psB.tile([128, JT, 2], f32, tag="psB", name="c2ps")
            c3ps = [
                psA.tile([128, CAP], f32, tag="psA", name=f"c3_{jt}")
                for jt in range(JT)
            ]

            def c3_r(r, start):
                for jt in range(JT):
                    nc.tensor.matmul(
                        c3ps[jt][:],
                        yt[:, 2 * r : 2 * r + 2, jt * 128 : (jt + 1) * 128],
                        xwt[:, 2 * r : 2 * r + 2, :],
                        start=start, stop=False, perf_mode=DR,
                        skip_group_check=True,
                    )

            def c2_r(r, start, stop):
                pt = r // 4
                for jt in range(JT):
                    nc.tensor.matmul(
                        c2ps[:, jt, :],
                        ysq[:, 2 * r : 2 * r + 2, jt * 128 : (jt + 1) * 128],
                        cs2[:, :, pt, :],
                        start=start, stop=stop, perf_mode=DR,
                        skip_group_check=True,
                    )

            # ysq as chunk-aligned quad ops, alternating DVE/ACT
            nc.vector.tensor_mul(ysq[:, 0:4, :], yt[:, 0:4, :], yt[:, 0:4, :])
            c3_r(0, True); c2_r(0, True, False)
            c3_r(1, False); c2_r(1, False, False)

            nc.scalar.square(ysq[:, 4:8, :], yt[:, 4:8, :])
            c3_r(2, False); c2_r(2, False, False)
            c3_r(3, False); c2_r(3, False, False)

            nc.vector.tensor_mul(ysq[:, 8:12, :], yt[:, 8:12, :], yt[:, 8:12, :])
            c3_r(4, False); c2_r(4, False, False)
            c3_r(5, False); c2_r(5, False, False)

            nc.scalar.square(ysq[:, 12:16, :], yt[:, 12:16, :])
            c3_r(6, False); c2_r(6, False, False)
            c3_r(7, False)
            # close the C3 groups with two K=1 fp16 C1 augmentations each
            # (these depend only on xk, so the psums close well before C2t)
            for jt in range(JT):
                nc.tensor.matmul(
                    c3ps[jt][:, 0:CPC], ones16[:], c1a[0:1, 0:CPC],
                    start=False, stop=False, skip_group_check=True,
                )
                nc.tensor.matmul(
                    c3ps[jt][:, CPC:CAP], ones16[:], c1b[0:1, CPC:CAP],
                    start=False, stop=True, skip_group_check=True,
                )
            c2_r(7, False, True)

            # C2t psum -> SBUF f32 (one small op)
            c2sb = pc.tile([128, JT, 2], f32, tag="c2sb")
            nc.gpsimd.tensor_copy(c2sb[:], c2ps[:])

            # ---- out: fp16 evac with per-partition C2 bias, 2 DMAs ----
            osb = py.tile([128, JT, CAP], f16, tag="osb")
            odv = out_d.rearrange("(jt l) n -> l jt n", l=128)

            def bias_evac(eng, jt, cl):
                dst = osb[:, jt, cl * CPC : (cl + 1) * CPC]
                src = c3ps[jt][:, cl * CPC : (cl + 1) * CPC]
                b = c2sb[:, jt, cl : cl + 1]
                if eng is nc.scalar:
                    eng.activation(dst, src, Ident, bias=b)
                else:
                    eng.tensor_scalar_add(dst, src, b)

            bias_evac(nc.vector, 0, 0); bias_evac(nc.scalar, 0, 1)
            bias_evac(nc.gpsimd, 1, 0); bias_evac(nc.vector, 1, 1)
            nc.sync.dma_start(odv[:, 0:2, :], osb[:, 0:2, :])
            bias_evac(nc.scalar, 2, 0); bias_evac(nc.gpsimd, 2, 1)
            bias_evac(nc.vector, 3, 0); bias_evac(nc.gpsimd, 3, 1)
            nc.scalar.dma_start(odv[:, 2:4, :], osb[:, 2:4, :])

    nc.compile()
    return nc


def kernel(X, Y, pi_dtw, classes):
    from concourse.bass_utils import run_bass_kernel_spmd

    X = np.asarray(X, dtype=np.float32)
    Y = np.asarray(Y, dtype=np.float32)
    pi_dtw = np.asarray(pi_dtw, dtype=np.float32)
    classes = np.asarray(classes).astype(np.int64)

    if "nc" not in _cache:
        _cache["nc"] = _build()
    nc = _cache["nc"]

    X8 = X.astype(FP8)
    Y8 = Y.astype(FP8)
    pi8 = pi_dtw.astype(FP8)
    idx = [np.nonzero(classes == c)[0] for c in range(C)]
    assert max(len(i) for i in idx) <= CPC, "class count exceeds capacity"

    # yt per Y half: [pp, (pt, d), j]
    yts = []
    for h in range(H):
        yh = Y8[h * NYH : (h + 1) * NYH]          # [j, p, d]
        a = yh.reshape(NYH, 2, 128, D).transpose(2, 1, 3, 0)  # [pp, pt, d, j]
        yts.append(np.ascontiguousarray(a.reshape(128, KC * NYH)))

    in_maps = []
    for k in range(NCORES):
        g, h = k >> 1, k & 1
        c0, c1 = 2 * g, 2 * g + 1
        xg = np.zeros((CAP, T, D), dtype=FP8)
        xg[0 : len(idx[c0])] = X8[idx[c0]]
        xg[CPC : CPC + len(idx[c1])] = X8[idx[c1]]
        # xk: [tp, d, tt, n]
        a = xg.reshape(CAP, 2, 128, D).transpose(2, 3, 1, 0)
        xk = np.ascontiguousarray(a.reshape(128, KC * CAP))
        # pis: pi [tp, c, tt, p] ++ piT [pp, c, pt, t]
        pg = pi8[[c0, c1]]                         # [c, t, p]
        b = pg.reshape(2, 2, 128, TP).transpose(2, 0, 1, 3)          # [tp,c,tt,p]
        bt = pg.reshape(2, TP, 2, 128).transpose(3, 0, 2, 1)         # [pp,c,pt,t]
        pik = np.concatenate(
            [b.reshape(128, -1), bt.reshape(128, -1)], axis=1
        )
        in_maps.append({"pis": np.ascontiguousarray(pik), "xk": xk, "yt": yts[h]})

    res = run_bass_kernel_spmd(nc, in_maps, core_ids=list(range(NCORES)))

    out = np.empty((N, NY), dtype=np.float32)
    for k in range(NCORES):
        g, h = k >> 1, k & 1
        blk = np.asarray(res.results[k]["outp"]).astype(np.float32)  # [j, n]
        jsel = slice(h * NYH, (h + 1) * NYH)
        c0, c1 = 2 * g, 2 * g + 1
        out[idx[c0], jsel] = blk[:, 0 : len(idx[c0])].T
        out[idx[c1], jsel] = blk[:, CPC : CPC + len(idx[c1])].T
    return out
